# revision 5
# baseline (speedup 1.0000x reference)
"""Contrastive (NT-Xent) loss kernel for TRN2, 8 NeuronCores.

Reference math: p = concat(proj_i, proj_j) [N=8192, D=128]; z = row-normalized p;
sim = z @ z.T; for each row r the logits are {sim[r, partner(r)]} U {sim[r, c]:
c != r, c != partner(r)} which is exactly {sim[r, c] : c != r}. So

    loss = -(1/N) * sum_r [ sim[r, partner(r)]/T - log(sum_{c != r} exp(sim[r, c]/T)) ]

with T = 0.5, partner(r) = (r + B) mod N. sim in [-1, 1] so exp(sim/T) in
[e^-2, e^2]: no max-subtraction needed for a stable logsumexp.

Sharding: data-parallel over rows. Each core gets the full p *rotated* by its
row base (np.roll), so a single SPMD program serves all cores: local rows are
always global rows R0..R0+1023 == local columns 0..1023, and the partner
diagonal always sits at local column offset 4096.

Per core: build zT [128(d) x 8192(n)] bf16 (normalize in row layout, DMA-xbar
transpose), then for each 128-row chunk m and 2048-col supergroup s:
4 bf16 matmuls -> PSUM [128, 2048] fp32 -> one ACT Exp(scale=2) with fused
accum_out row-sums. The partner diagonal is pulled from the exp'd tile with an
identity mask (tensor_tensor_reduce). Row loss = ln(exp(2*pos) / (L - e^2)).
The diagonal term exp(2*sim[r,r]) is removed as the constant e^2 (bf16 z rows
have |z|^2 = 1 +- ~3e-4; the induced error on the loss is ~1e-6 relative).

Host finishes: loss = -sum(all per-row values) / N.
"""

import numpy as np

import concourse.bass as bass
import concourse.mybir as mybir
import concourse.tile as tile
from concourse import bacc
from concourse.bass_utils import run_bass_kernel_spmd
from concourse.masks import make_identity

B = 4096
D = 128
N = 2 * B
NCORES = 8
ROWS = N // NCORES          # 1024 rows per core
P = 128
CHUNKS = ROWS // P          # 8 row chunks per core
SG = 4                      # column supergroups of 2048
SG_COLS = N // SG           # 2048
G = 16                      # zT column groups of 512
GCOLS = N // G              # 512
NT = N // P                 # 64 source p tiles
E2 = float(np.exp(np.float64(2.0)))  # exp(sim[r,r]/T) with sim[r,r] = 1

f32 = mybir.dt.float32
bf16 = mybir.dt.bfloat16
Alu = mybir.AluOpType
Act = mybir.ActivationFunctionType


def _build_kernel(tc: tile.TileContext, out_ap: bass.AP, pc_ap: bass.AP):
    nc = tc.nc
    with (
        tc.tile_pool(name="zt", bufs=1) as ztp,
        tc.tile_pool(name="io", bufs=8) as iop,
        tc.tile_pool(name="tmp", bufs=2) as tmp,
        tc.tile_pool(name="small", bufs=1) as smallp,
        tc.tile_pool(name="es", bufs=2) as esp,
        tc.tile_pool(name="ps", bufs=2, space="PSUM") as psp,
    ):
        ident = smallp.tile([P, P], bf16, tag="ident")
        make_identity(nc, ident[:])

        # zT column groups: zT[d, n] = z[n, d], bf16, 16 groups of 512 cols.
        ztg = [
            ztp.tile([P, GCOLS], bf16, tag=f"ztg{g}", name=f"ztg{g}")
            for g in range(G)
        ]

        sums = smallp.tile([P, CHUNKS * SG], f32, tag="sums")  # col = m*SG + s
        expo = smallp.tile([P, CHUNKS], f32, tag="expo")       # exp(2*pos)

        # ---- preamble: normalize rows, build zT ----
        # batches of 8 source tiles (= 2 zT groups); 4 tiles share one DMA /
        # tensor_mul / reduce.  (tensor_tensor_reduce crashes on HW; avoid.)
        for b in range(NT // 8):
            ssq = tmp.tile([P, 8], f32, tag="ssq")
            pt4s = []
            for h in range(2):
                rows0 = (8 * b + 4 * h) * P
                pt4 = iop.tile([P, 4, D], f32, tag="pt4")
                nc.sync.dma_start(
                    pt4[:],
                    pc_ap[rows0:rows0 + 4 * P, :].rearrange("(u p) d -> p u d", p=P),
                )
                sq4 = tmp.tile([P, 4, D], f32, tag="sq4")
                nc.vector.tensor_mul(sq4[:], pt4[:], pt4[:])
                nc.vector.reduce_sum(
                    ssq[:, 4 * h:4 * h + 4], sq4[:], axis=mybir.AxisListType.X
                )
                pt4s.append(pt4)
            # rnorm = 1/sqrt(ssq) = exp(-0.5*ln(ssq)); ln/exp share one ACT
            # table set (natural_log_exp_and_others), unlike Sqrt.
            lnssq = tmp.tile([P, 8], f32, tag="lnssq")
            nc.scalar.activation(lnssq[:], ssq[:], Act.Ln)
            rnorm = tmp.tile([P, 8], f32, tag="rnorm")
            nc.scalar.activation(rnorm[:], lnssq[:], Act.Exp, scale=-0.5)
            for h in range(2):
                for j in range(4):
                    t = 8 * b + 4 * h + j
                    zt_row = iop.tile([P, D], bf16, tag="ztile")
                    nc.vector.tensor_scalar_mul(
                        zt_row[:], pt4s[h][:, j, :], rnorm[:, 4 * h + j:4 * h + j + 1]
                    )
                    g, k = t // 4, t % 4
                    nc.sync.dma_start_transpose(ztg[g][:, k * P:(k + 1) * P], zt_row[:])

        # ---- main loop: S tiles, exp, row sums ----
        for s in range(SG):
            for m in range(CHUNKS):
                ps = psp.tile([P, SG_COLS], f32, tag="ps")
                lhsT = ztg[m // 4][:, (m % 4) * P:(m % 4 + 1) * P]
                for k in range(4):
                    nc.tensor.matmul(
                        ps[:, k * GCOLS:(k + 1) * GCOLS],
                        lhsT, ztg[4 * s + k][:], start=True, stop=True,
                    )
                es = esp.tile([P, SG_COLS], bf16, tag="es")
                nc.scalar.activation(
                    es[:], ps[:], Act.Exp, scale=2.0,
                    accum_out=sums[:, m * SG + s:m * SG + s + 1],
                )
                if s == 2:
                    # partner diagonal: local cols 4096 + m*128 + i -> within
                    # supergroup 2 at offset m*128.
                    sq2 = tmp.tile([P, P], bf16, tag="sq2")
                    nc.vector.tensor_mul(sq2[:], es[:, m * P:(m + 1) * P], ident[:])
                    nc.vector.reduce_sum(
                        expo[:, m:m + 1], sq2[:], axis=mybir.AxisListType.X
                    )

        # ---- tail: per-row loss = ln(expo) - ln(L) = ln(expo / L) ----
        ltot = smallp.tile([P, CHUNKS], f32, tag="ltot")
        nc.vector.reduce_sum(
            ltot[:], sums[:].rearrange("p (m s) -> p m s", s=SG),
            axis=mybir.AxisListType.X,
        )
        nc.vector.tensor_scalar_add(ltot[:], ltot[:], -E2)
        rcp = smallp.tile([P, CHUNKS], f32, tag="rcp")
        nc.vector.reciprocal(rcp[:], ltot[:])
        ratio = smallp.tile([P, CHUNKS], f32, tag="ratio")
        nc.vector.tensor_mul(ratio[:], expo[:], rcp[:])
        res = smallp.tile([P, CHUNKS], f32, tag="res")
        nc.scalar.activation(res[:], ratio[:], Act.Ln)
        nc.sync.dma_start(out_ap[:, :], res[:])


_CACHE: dict = {}


def _compiled():
    if "nc" not in _CACHE:
        nc = bacc.Bacc(
            "TRN2", target_bir_lowering=False, debug=False,
            enable_asserts=True, num_devices=NCORES,
        )
        pc = nc.dram_tensor("pc", [N, D], f32, kind="ExternalInput").ap()
        out = nc.dram_tensor("partial", [P, CHUNKS], f32, kind="ExternalOutput").ap()
        with tile.TileContext(nc) as tc:
            _build_kernel(tc, out, pc)
        nc.compile()
        _CACHE["nc"] = nc
    return _CACHE["nc"]


def kernel(proj_i: np.ndarray, proj_j: np.ndarray, **run_kwargs) -> np.ndarray:
    assert proj_i.shape == (B, D) and proj_j.shape == (B, D)
    nc = _compiled()
    p = np.concatenate(
        [np.asarray(proj_i, np.float32), np.asarray(proj_j, np.float32)], axis=0
    )
    in_maps = [
        {"pc": np.ascontiguousarray(np.roll(p, -c * ROWS, axis=0))}
        for c in range(NCORES)
    ]
    res = run_bass_kernel_spmd(nc, in_maps, list(range(NCORES)), **run_kwargs)
    total = sum(
        np.asarray(r["partial"], np.float64).sum() for r in res.results
    )
    _CACHE["last_results"] = res
    return np.float32(-total / N)


# revision 6
# speedup vs baseline: 1.3592x; 1.3592x over previous
"""Contrastive (NT-Xent) loss kernel for TRN2, 8 NeuronCores.

Reference math: p = concat(proj_i, proj_j) [N=8192, D=128]; z = row-normalized p;
sim = z @ z.T; for each row r the logits are {sim[r, partner(r)]} U {sim[r, c]:
c != r, c != partner(r)} which is exactly {sim[r, c] : c != r}. So

    loss = -(1/N) * sum_r [ sim[r, partner(r)]/T - log(sum_{c != r} exp(sim[r, c]/T)) ]

with T = 0.5, partner(r) = (r + B) mod N. sim in [-1, 1] so exp(sim/T) in
[e^-2, e^2]: no max-subtraction needed for a stable logsumexp.

Sharding: data-parallel over rows. Each core gets the full p *rotated* by its
row base (np.roll), so a single SPMD program serves all cores: local rows are
always global rows R0..R0+1023 == local columns 0..1023, and the partner
diagonal always sits at local column offset 4096.

Per core:
 1. Normalize rows in [128 x D] row layout (sumsq via tensor_mul+reduce —
    tensor_tensor_reduce crashes on HW; rsqrt via exp(-0.5*ln), since the
    Rsqrt activation is banned and ln/exp share an ACT table set).
 2. Stage z (bf16) to a DRAM scratch, then build zT [128(d) x 8192(n)] with
    4 big DRAM->SBUF xbar transposes (64 small SBUF->SBUF transposes
    serialize on the Sync engine — measured 76us; this is ~8us).
 3. Main loop over 4 column supergroups x 8 row chunks: 4 bf16 matmuls into
    a [128, 2048] 4-bank PSUM tile -> one ACT Exp(scale=2) with fused
    accum_out row-sum. Partner diagonal pulled from the exp'd tile with an
    identity mask (supergroup 2 only).
 4. Output per-row exp(2*pos) and raw rowsum-of-exp; host finishes with
    loss_row = ln(expo) - ln(rowsum - e^2) and the global mean. The diagonal
    exp(2*sim[r,r]) is removed as the constant e^2 (bf16 z rows have
    |z|^2 = 1 +- ~3e-4; induced loss error ~1e-6 relative).
"""

import numpy as np

import concourse.bass as bass
import concourse.mybir as mybir
import concourse.tile as tile
from concourse import bacc
from concourse.bass_utils import run_bass_kernel_spmd
from concourse.masks import make_identity

B = 4096
D = 128
N = 2 * B
NCORES = 8
ROWS = N // NCORES          # 1024 rows per core
P = 128
CHUNKS = ROWS // P          # 8 row chunks per core
SG = 4                      # column supergroups of 2048
SG_COLS = N // SG           # 2048
NT = N // P                 # 64 source p tiles
E2 = float(np.exp(np.float64(2.0)))  # exp(sim[r,r]/T) with sim[r,r] = 1

f32 = mybir.dt.float32
bf16 = mybir.dt.bfloat16
Alu = mybir.AluOpType
Act = mybir.ActivationFunctionType
AxX = mybir.AxisListType.X


def _build_kernel(tc: tile.TileContext, out_ap: bass.AP, pc_ap: bass.AP):
    nc = tc.nc
    with (
        tc.tile_pool(name="zt", bufs=1) as ztp,
        tc.tile_pool(name="io", bufs=10) as iop,
        tc.tile_pool(name="zo", bufs=4) as zop,
        tc.tile_pool(name="tmp", bufs=2) as tmp,
        tc.tile_pool(name="small", bufs=1) as smallp,
        tc.tile_pool(name="es", bufs=2) as esp,
        tc.tile_pool(name="ps", bufs=2, space="PSUM") as psp,
        tc.tile_pool(name="zd", bufs=1, space="DRAM") as zdp,
    ):
        ident = smallp.tile([P, P], bf16, tag="ident")
        make_identity(nc, ident[:])

        # zT supergroups: zT[d, n] = z[n, d], bf16, 4 groups of 2048 cols.
        ztg = [
            ztp.tile([P, SG_COLS], bf16, tag=f"ztg{s}", name=f"ztg{s}")
            for s in range(SG)
        ]
        # DRAM staging for z rows (bf16), one tensor per supergroup so each
        # big transpose only waits on its own 4 stores.
        zdram = [
            zdp.tile([SG_COLS, D], bf16, tag=f"zd{s}", name=f"zd{s}")
            for s in range(SG)
        ]

        sums = smallp.tile([P, CHUNKS * SG], f32, tag="sums")  # col = m*SG + s
        expo = smallp.tile([P, CHUNKS], f32, tag="expo")       # exp(2*pos)
        ssq = smallp.tile([P, NT], f32, tag="ssq")
        lnr = smallp.tile([P, NT], f32, tag="lnr")
        rnorm = smallp.tile([P, NT], f32, tag="rnorm")

        # ---- preamble: normalize rows, stage z to DRAM, big transposes ----
        # two halves of 32 tiles; one Ln + one Exp per half (ACT table sets
        # reload on every Ln<->Exp switch, so batch the norm scalars).
        for h in range(2):
            pt4s = {}
            for q in range(8):
                tb = 8 * h + q          # batch of 4 source tiles
                rows0 = 4 * tb * P
                pt4 = iop.tile([P, 4, D], f32, tag="pt4", name=f"pt4_{tb}")
                nc.sync.dma_start(
                    pt4[:],
                    pc_ap[rows0:rows0 + 4 * P, :].rearrange("(u p) d -> p u d", p=P),
                )
                sq4 = tmp.tile([P, 4, D], f32, tag="sq4")
                nc.vector.tensor_mul(sq4[:], pt4[:], pt4[:])
                nc.vector.reduce_sum(ssq[:, 4 * tb:4 * tb + 4], sq4[:], axis=AxX)
                pt4s[tb] = pt4
            nc.scalar.activation(
                lnr[:, 32 * h:32 * h + 32], ssq[:, 32 * h:32 * h + 32], Act.Ln
            )
            nc.scalar.activation(
                rnorm[:, 32 * h:32 * h + 32], lnr[:, 32 * h:32 * h + 32],
                Act.Exp, scale=-0.5,
            )
            for q in range(8):
                tb = 8 * h + q
                s, w = tb // 4, tb % 4   # supergroup, slot within supergroup
                zt4 = zop.tile([P, 4, D], bf16, tag="zt4")
                for j in range(4):
                    t = 4 * tb + j
                    nc.vector.tensor_scalar_mul(
                        zt4[:, j, :], pt4s[tb][:, j, :], rnorm[:, t:t + 1]
                    )
                nc.sync.dma_start(
                    zdram[s][512 * w:512 * (w + 1), :]
                    .rearrange("(u p) d -> p u d", p=P),
                    zt4[:],
                )
                if w == 3:
                    nc.sync.dma_start_transpose(ztg[s][:], zdram[s][:])

        # ---- main loop: S tiles, exp, row sums ----
        for s in range(SG):
            for m in range(CHUNKS):
                ps = psp.tile([P, SG_COLS], f32, tag="ps")
                lhsT = ztg[0][:, m * P:(m + 1) * P]
                for k in range(4):
                    nc.tensor.matmul(
                        ps[:, 512 * k:512 * (k + 1)],
                        lhsT, ztg[s][:, 512 * k:512 * (k + 1)],
                        start=True, stop=True,
                    )
                es = esp.tile([P, SG_COLS], bf16, tag="es")
                nc.scalar.activation(
                    es[:], ps[:], Act.Exp, scale=2.0,
                    accum_out=sums[:, m * SG + s:m * SG + s + 1],
                )
                if s == 2:
                    # partner diagonal: local cols 4096 + m*128 + i -> within
                    # supergroup 2 at offset m*128.
                    sq2 = tmp.tile([P, P], bf16, tag="sq2")
                    nc.vector.tensor_mul(sq2[:], es[:, m * P:(m + 1) * P], ident[:])
                    nc.vector.reduce_sum(expo[:, m:m + 1], sq2[:], axis=AxX)

        # ---- tail: ship per-row expo and raw rowsums; host does the logs ----
        lsum = smallp.tile([P, CHUNKS], f32, tag="lsum")
        nc.vector.reduce_sum(
            lsum[:], sums[:].rearrange("p (m s) -> p m s", s=SG), axis=AxX
        )
        nc.sync.dma_start(out_ap[:, 0:CHUNKS], expo[:])
        nc.sync.dma_start(out_ap[:, CHUNKS:2 * CHUNKS], lsum[:])


_CACHE: dict = {}


def _compiled():
    if "nc" not in _CACHE:
        nc = bacc.Bacc(
            "TRN2", target_bir_lowering=False, debug=False,
            enable_asserts=True, num_devices=NCORES,
        )
        pc = nc.dram_tensor("pc", [N, D], f32, kind="ExternalInput").ap()
        out = nc.dram_tensor(
            "partial", [P, 2 * CHUNKS], f32, kind="ExternalOutput"
        ).ap()
        with tile.TileContext(nc) as tc:
            _build_kernel(tc, out, pc)
        nc.compile()
        _CACHE["nc"] = nc
    return _CACHE["nc"]


def kernel(proj_i: np.ndarray, proj_j: np.ndarray, **run_kwargs) -> np.ndarray:
    assert proj_i.shape == (B, D) and proj_j.shape == (B, D)
    nc = _compiled()
    p = np.concatenate(
        [np.asarray(proj_i, np.float32), np.asarray(proj_j, np.float32)], axis=0
    )
    in_maps = [
        {"pc": np.ascontiguousarray(np.roll(p, -c * ROWS, axis=0))}
        for c in range(NCORES)
    ]
    res = run_bass_kernel_spmd(nc, in_maps, list(range(NCORES)), **run_kwargs)
    total = 0.0
    for r in res.results:
        part = np.asarray(r["partial"], np.float64)
        expo, lsum = part[:, :CHUNKS], part[:, CHUNKS:]
        total += (np.log(expo) - np.log(lsum - E2)).sum()
    _CACHE["last_results"] = res
    return np.float32(-total / N)


# revision 8
# speedup vs baseline: 1.5351x; 1.1294x over previous
"""Contrastive (NT-Xent) loss kernel for TRN2, 8 NeuronCores.

Reference math: p = concat(proj_i, proj_j) [N=8192, D=128]; z = row-normalized p;
sim = z @ z.T; for each row r the logits are {sim[r, partner(r)]} U {sim[r, c]:
c != r, c != partner(r)} which is exactly {sim[r, c] : c != r}. So

    loss = -(1/N) * sum_r [ sim[r, partner(r)]/T - log(sum_{c != r} exp(sim[r, c]/T)) ]

with T = 0.5, partner(r) = (r + B) mod N. sim in [-1, 1] so exp(sim/T) in
[e^-2, e^2]: no max-subtraction needed for a stable logsumexp.

Sharding: data-parallel over rows. Each core gets the full p *rotated* by its
row base (np.roll), so a single SPMD program serves all cores: local rows are
always global rows R0..R0+1023 == local columns 0..1023, and the partner
diagonal always sits at local column offset 4096.

Per core:
 1. Normalize rows in [128 x D] row layout (sumsq via tensor_mul+reduce —
    tensor_tensor_reduce crashes on HW; rsqrt via exp(-0.5*ln), since the
    Rsqrt activation is banned and ln/exp share an ACT table set).
 2. Stage z (bf16) to a DRAM scratch, then build zT [128(d) x 8192(n)] with
    4 big DRAM->SBUF xbar transposes (64 small SBUF->SBUF transposes
    serialize on the Sync engine — measured 76us; this is ~8us).
 3. Main loop over 4 column supergroups x 8 row chunks: 4 bf16 matmuls into
    a [128, 2048] 4-bank PSUM tile -> one ACT Exp(scale=2) with fused
    accum_out row-sum. Partner diagonal pulled from the exp'd tile with an
    identity mask (supergroup 2 only).
 4. Output per-row exp(2*pos) and raw rowsum-of-exp; host finishes with
    loss_row = ln(expo) - ln(rowsum - e^2) and the global mean. The diagonal
    exp(2*sim[r,r]) is removed as the constant e^2 (bf16 z rows have
    |z|^2 = 1 +- ~3e-4; induced loss error ~1e-6 relative).
"""

import numpy as np

import concourse.bass as bass
import concourse.mybir as mybir
import concourse.tile as tile
from concourse import bacc
from concourse.bass_utils import run_bass_kernel_spmd
from concourse.masks import make_identity

B = 4096
D = 128
N = 2 * B
NCORES = 8
ROWS = N // NCORES          # 1024 rows per core
P = 128
CHUNKS = ROWS // P          # 8 row chunks per core
SG = 4                      # column supergroups of 2048
SG_COLS = N // SG           # 2048
NT = N // P                 # 64 source p tiles
E2 = float(np.exp(np.float64(2.0)))  # exp(sim[r,r]/T) with sim[r,r] = 1

f32 = mybir.dt.float32
bf16 = mybir.dt.bfloat16
Alu = mybir.AluOpType
Act = mybir.ActivationFunctionType
AxX = mybir.AxisListType.X


def _build_kernel(tc: tile.TileContext, out_ap: bass.AP, pc_ap: bass.AP):
    nc = tc.nc
    with (
        tc.tile_pool(name="zt", bufs=1) as ztp,
        tc.tile_pool(name="io", bufs=16) as iop,
        tc.tile_pool(name="zo", bufs=4) as zop,
        tc.tile_pool(name="tmp", bufs=2) as tmp,
        tc.tile_pool(name="small", bufs=1) as smallp,
        tc.tile_pool(name="es", bufs=2) as esp,
        tc.tile_pool(name="ps", bufs=2, space="PSUM") as psp,
        tc.tile_pool(name="zd", bufs=1, space="DRAM") as zdp,
    ):
        ident = smallp.tile([P, P], bf16, tag="ident")
        make_identity(nc, ident[:])

        # zT supergroups: zT[d, n] = z[n, d], bf16, 4 groups of 2048 cols.
        ztg = [
            ztp.tile([P, SG_COLS], bf16, tag=f"ztg{s}", name=f"ztg{s}")
            for s in range(SG)
        ]
        # DRAM staging for z rows (bf16), one tensor per supergroup so each
        # big transpose only waits on its own 4 stores.
        zdram = [
            zdp.tile([SG_COLS, D], bf16, tag=f"zd{s}", name=f"zd{s}")
            for s in range(SG)
        ]

        sums = smallp.tile([P, CHUNKS * SG], f32, tag="sums")  # col = m*SG + s
        expo = smallp.tile([P, CHUNKS], f32, tag="expo")       # exp(2*pos)
        ssq = smallp.tile([P, NT], f32, tag="ssq")
        lnr = smallp.tile([P, NT], f32, tag="lnr")
        rnorm = smallp.tile([P, NT], f32, tag="rnorm")

        # ---- preamble: normalize rows, stage z to DRAM, big transposes ----
        # Prefetch every input tile first (4 MB fits in SBUF), then compute
        # norms in groups [16, 48]: the small first group gets supergroup 0's
        # z built fast so matmul+exp start early; the big second group's
        # norms hide under the s=0 exp stream. One Ln+Exp pair per group
        # (each Ln<->Exp switch reloads ACT tables, ~1.3us each).
        pt4s = {}
        for tb in range(16):
            rows0 = 4 * tb * P
            pt4 = iop.tile([P, 4, D], f32, tag="pt4", name=f"pt4_{tb}")
            nc.sync.dma_start(
                pt4[:],
                pc_ap[rows0:rows0 + 4 * P, :].rearrange("(u p) d -> p u d", p=P),
            )
            pt4s[tb] = pt4
        for g0, g1 in ((0, 4), (4, 16)):     # batch-of-4 ranges per norm group
            for tb in range(g0, g1):
                sq4 = tmp.tile([P, 4, D], f32, tag="sq4")
                nc.vector.tensor_mul(sq4[:], pt4s[tb][:], pt4s[tb][:])
                nc.vector.reduce_sum(ssq[:, 4 * tb:4 * tb + 4], sq4[:], axis=AxX)
            c0, c1 = 4 * g0, 4 * g1
            nc.scalar.activation(lnr[:, c0:c1], ssq[:, c0:c1], Act.Ln)
            nc.scalar.activation(
                rnorm[:, c0:c1], lnr[:, c0:c1], Act.Exp, scale=-0.5
            )
            for tb in range(g0, g1):
                s, w = tb // 4, tb % 4   # supergroup, slot within supergroup
                zt4 = zop.tile([P, 4, D], bf16, tag="zt4")
                for j in range(4):
                    t = 4 * tb + j
                    nc.vector.tensor_scalar_mul(
                        zt4[:, j, :], pt4s[tb][:, j, :], rnorm[:, t:t + 1]
                    )
                nc.sync.dma_start(
                    zdram[s][512 * w:512 * (w + 1), :]
                    .rearrange("(u p) d -> p u d", p=P),
                    zt4[:],
                )
                if w == 3:
                    nc.sync.dma_start_transpose(ztg[s][:], zdram[s][:])

        # ---- main loop: S tiles, exp, row sums ----
        for s in range(SG):
            for m in range(CHUNKS):
                ps = psp.tile([P, SG_COLS], f32, tag="ps")
                lhsT = ztg[0][:, m * P:(m + 1) * P]
                for k in range(4):
                    nc.tensor.matmul(
                        ps[:, 512 * k:512 * (k + 1)],
                        lhsT, ztg[s][:, 512 * k:512 * (k + 1)],
                        start=True, stop=True,
                    )
                es = esp.tile([P, SG_COLS], bf16, tag="es")
                nc.scalar.activation(
                    es[:], ps[:], Act.Exp, scale=2.0,
                    accum_out=sums[:, m * SG + s:m * SG + s + 1],
                )
                if s == 2:
                    # partner diagonal: local cols 4096 + m*128 + i -> within
                    # supergroup 2 at offset m*128.
                    sq2 = tmp.tile([P, P], bf16, tag="sq2")
                    nc.vector.tensor_mul(sq2[:], es[:, m * P:(m + 1) * P], ident[:])
                    nc.vector.reduce_sum(expo[:, m:m + 1], sq2[:], axis=AxX)

        # ---- tail: ship per-row expo and raw rowsums; host does the logs ----
        lsum = smallp.tile([P, CHUNKS], f32, tag="lsum")
        nc.vector.reduce_sum(
            lsum[:], sums[:].rearrange("p (m s) -> p m s", s=SG), axis=AxX
        )
        nc.sync.dma_start(out_ap[:, 0:CHUNKS], expo[:])
        nc.sync.dma_start(out_ap[:, CHUNKS:2 * CHUNKS], lsum[:])


_CACHE: dict = {}


def _compiled():
    if "nc" not in _CACHE:
        nc = bacc.Bacc(
            "TRN2", target_bir_lowering=False, debug=False,
            enable_asserts=True, num_devices=NCORES,
        )
        pc = nc.dram_tensor("pc", [N, D], f32, kind="ExternalInput").ap()
        out = nc.dram_tensor(
            "partial", [P, 2 * CHUNKS], f32, kind="ExternalOutput"
        ).ap()
        with tile.TileContext(nc) as tc:
            _build_kernel(tc, out, pc)
        nc.compile()
        _CACHE["nc"] = nc
    return _CACHE["nc"]


def kernel(proj_i: np.ndarray, proj_j: np.ndarray, **run_kwargs) -> np.ndarray:
    assert proj_i.shape == (B, D) and proj_j.shape == (B, D)
    nc = _compiled()
    p = np.concatenate(
        [np.asarray(proj_i, np.float32), np.asarray(proj_j, np.float32)], axis=0
    )
    in_maps = [
        {"pc": np.ascontiguousarray(np.roll(p, -c * ROWS, axis=0))}
        for c in range(NCORES)
    ]
    res = run_bass_kernel_spmd(nc, in_maps, list(range(NCORES)), **run_kwargs)
    total = 0.0
    for r in res.results:
        part = np.asarray(r["partial"], np.float64)
        expo, lsum = part[:, :CHUNKS], part[:, CHUNKS:]
        total += (np.log(expo) - np.log(lsum - E2)).sum()
    _CACHE["last_results"] = res
    return np.float32(-total / N)
